# revision 1
# baseline (speedup 1.0000x reference)
"""HF OpenMoe attention (B=2,S=2048,HID=2048,NH=16,NKV=4,HD=128) on 8 trn2 cores.

Sharding: core c -> (batch b=c//4, kv-group g=c%4). Each core computes Q/K/V
projections for its 4 query heads + 1 kv head, RoPE, causal flash attention in
S^T layout (scores transposed: [k, q], softmax over the partition dim via
ones-matmul), and its partial o_proj; a 4-way ReduceScatter sums the o_proj
partials, each core returning a 512-row slice of o^T for its batch.

All matmuls run as float32r (full PE rate at N=512).
"""
import numpy as np
import concourse.bass as bass
import concourse.bacc as bacc
import concourse.tile as tile
import concourse.mybir as mybir
from concourse.bass_utils import run_bass_kernel_spmd
from concourse.masks import make_identity

f32 = mybir.dt.float32
f32r = mybir.dt.float32r
AF = mybir.ActivationFunctionType
MUL = mybir.AluOpType.mult
ADD = mybir.AluOpType.add

B, S, HID = 2, 2048, 2048
NH, NKV, HD = 16, 4, 128
GH = NH // NKV          # query heads per core (4)
TB = 512                # token block (q block / projection block)
NT = S // TB            # 4 token blocks
NCT = HID // 128        # 16 contraction tiles
NKT = S // 128          # 16 key tiles

_CACHE = {}


def _build(causal: bool, with_rs: bool = True):
    nc = bacc.Bacc("TRN2", target_bir_lowering=False, debug=False, num_devices=8)
    xt = nc.dram_tensor("xt", [HID, S], f32, kind="ExternalInput").ap()
    wq = nc.dram_tensor("wq", [HID, GH * HD], f32, kind="ExternalInput").ap()
    wk = nc.dram_tensor("wk", [HID, HD], f32, kind="ExternalInput").ap()
    wv = nc.dram_tensor("wv", [HID, HD], f32, kind="ExternalInput").ap()
    wo = nc.dram_tensor("wo", [GH * HD, HID], f32, kind="ExternalInput").ap()
    cos_d = nc.dram_tensor("cos_t", [HD, S], f32, kind="ExternalInput").ap()
    sin_d = nc.dram_tensor("sin_m", [HD, S], f32, kind="ExternalInput").ap()
    cm_d = nc.dram_tensor("cmask", [128, 4 * TB], f32, kind="ExternalInput").ap()
    id_d = nc.dram_tensor("ident_in", [128, 128], f32, kind="ExternalInput").ap()
    on_d = nc.dram_tensor("ones_in", [128, 128], f32, kind="ExternalInput").ap()
    out_r = nc.dram_tensor("out_r", [TB, S], f32, kind="ExternalOutput").ap()

    with tile.TileContext(nc) as tc:
        with (
            tc.tile_pool(name="glob", bufs=1) as glob,
            tc.tile_pool(name="dram", bufs=1, space="DRAM") as dram,
        ):
            # ---- global resident stores (65 KB/partition) ----
            kt_rope = glob.tile([128, S], f32r, tag="kt")          # roped K^T [d, k]
            v_all = glob.tile([128, S], f32r, tag="v")             # V natural, 128i:+128 = tile i
            qt_rope = [glob.tile([128, S], f32r, tag=f"q{h}", name=f"qt_rope{h}")
                       for h in range(GH)]
            ident = glob.tile([128, 128], f32r, tag="ident")
            nc.sync.dma_start(ident[:], id_d[:].bitcast(f32r))
            ones = glob.tile([128, 128], f32r, tag="ones")
            nc.sync.dma_start(ones[:], on_d[:].bitcast(f32r))

            oT_part = dram.tile([HID, S], f32)                     # o^T partial
            oT_red = dram.tile([TB, S], f32)

            # ---- phase A: projections + rope (phase-scoped SBUF) ----
            with tc.tile_pool(name="pA", bufs=1) as pA, \
                 tc.tile_pool(name="psA", bufs=1, space="PSUM") as psA:
                wq_all = pA.tile([128, NCT * GH * HD], f32r, tag="wq")   # [c-sub, ci*512+d]
                wk_all = pA.tile([128, NCT * HD], f32r, tag="wk")
                wv_all = pA.tile([128, NCT * HD], f32r, tag="wv")
                for ci in range(NCT):
                    cs_ = slice(128 * ci, 128 * (ci + 1))
                    nc.sync.dma_start(wq_all[:, ci * 512:(ci + 1) * 512], wq[cs_, :].bitcast(f32r))
                    nc.sync.dma_start(wk_all[:, ci * 128:(ci + 1) * 128], wk[cs_, :].bitcast(f32r))
                    nc.sync.dma_start(wv_all[:, ci * 128:(ci + 1) * 128], wv[cs_, :].bitcast(f32r))
                cos_s = pA.tile([128, S], f32, tag="cos")
                sin_s = pA.tile([128, S], f32, tag="sin")
                nc.sync.dma_start(cos_s[:], cos_d[:])
                nc.sync.dma_start(sin_s[:], sin_d[:])

                def rope(ps, dst_ap, tb):
                    """dst = ps*cos + swap64(ps)*sin_mod for token block tb."""
                    cs = cos_s[:, TB * tb:TB * (tb + 1)]
                    sn = sin_s[:, TB * tb:TB * (tb + 1)]
                    raw = pA.tile([128, TB], f32, tag="raw", bufs=3, name="raw")
                    nc.vector.tensor_copy(raw[:], ps[:])
                    rot = pA.tile([128, TB], f32, tag="rot", bufs=3, name="rot")
                    nc.sync.dma_start(rot[0:64, :], raw[64:128, :])
                    nc.sync.dma_start(rot[64:128, :], raw[0:64, :])
                    m1 = pA.tile([128, TB], f32, tag="m1", bufs=3, name="m1")
                    nc.vector.tensor_tensor(m1[:], raw[:], cs, op=MUL)
                    m2 = pA.tile([128, TB], f32, tag="m2", bufs=3, name="m2")
                    nc.vector.tensor_tensor(m2[:], rot[:], sn, op=MUL)
                    nc.vector.tensor_tensor(dst_ap, m1[:], m2[:], op=ADD)

                for tb in range(NT):
                    xt_t = []
                    for ci in range(NCT):
                        t = pA.tile([128, TB], f32r, tag="xt", bufs=16, name="xt")
                        nc.sync.dma_start(
                            t[:], xt[128 * ci:128 * (ci + 1),
                                     TB * tb:TB * (tb + 1)].bitcast(f32r))
                        xt_t.append(t)

                    ps_k = psA.tile([128, TB], f32, tag="pk")
                    ps_vt = psA.tile([128, TB], f32, tag="pv")
                    ps_q = [psA.tile([128, TB], f32, tag=f"pq{h}", name=f"ps_q{h}")
                            for h in range(GH)]
                    for ci in range(NCT):
                        st, sp = ci == 0, ci == NCT - 1
                        nc.tensor.matmul(ps_k[:], wk_all[:, ci * 128:(ci + 1) * 128],
                                         xt_t[ci][:], start=st, stop=sp)
                        nc.tensor.matmul(ps_vt[:], wv_all[:, ci * 128:(ci + 1) * 128],
                                         xt_t[ci][:], start=st, stop=sp)
                        for h in range(GH):
                            nc.tensor.matmul(ps_q[h][:],
                                             wq_all[:, ci * 512 + 128 * h:ci * 512 + 128 * (h + 1)],
                                             xt_t[ci][:], start=st, stop=sp)

                    rope(ps_k, kt_rope[:, TB * tb:TB * (tb + 1)], tb)
                    for h in range(GH):
                        rope(ps_q[h], qt_rope[h][:, TB * tb:TB * (tb + 1)], tb)

                    # V: V^T to sbuf, then PE-transpose 128-col pieces to natural layout
                    vt_sb = pA.tile([128, TB], f32r, tag="vts", bufs=2, name="vt_sb")
                    nc.vector.tensor_copy(vt_sb[:], ps_vt[:])
                    for u in range(TB // 128):
                        ps_tr = psA.tile([128, 128], f32r, tag="ptr", bufs=2, name="ps_tr")
                        nc.tensor.transpose(ps_tr[:], vt_sb[:, 128 * u:128 * (u + 1)], ident[:])
                        nc.vector.tensor_copy(
                            v_all[:, 128 * (4 * tb + u):128 * (4 * tb + u + 1)], ps_tr[:])

            # ---- phase B: attention + partial o_proj (phase-scoped SBUF) ----
            with tc.tile_pool(name="pB", bufs=1) as pB, \
                 tc.tile_pool(name="psB", bufs=1, space="PSUM") as psB:
                wo_all = pB.tile([128, GH * HID], f32r, tag="wo")  # [j-sub, jh*2048+c]
                for jh in range(GH):
                    nc.sync.dma_start(wo_all[:, jh * HID:(jh + 1) * HID],
                                      wo[128 * jh:128 * (jh + 1), :].bitcast(f32r))
                cm_s = pB.tile([128, 4 * TB], f32, tag="cm")
                nc.sync.dma_start(cm_s[:], cm_d[:])

                for j in range(NT):
                    nkt = 4 * (j + 1) if causal else NKT
                    ps_o = [psB.tile([128, TB], f32, tag=f"po{h}", name=f"ps_o{h}")
                            for h in range(GH)]
                    acc = [pB.tile([128, TB], f32r, tag=f"acc{h}", bufs=2, name=f"acch{h}")
                           for h in range(GH)]
                    for i in range(nkt):
                        m = i - 4 * j if causal else -1
                        for h in range(GH):
                            ps_s = psB.tile([128, TB], f32, tag="ps_s", bufs=2, name="ps_s")
                            nc.tensor.matmul(ps_s[:], kt_rope[:, 128 * i:128 * (i + 1)],
                                             qt_rope[h][:, TB * j:TB * (j + 1)],
                                             start=True, stop=True)
                            pt = pB.tile([128, TB], f32r, tag="pt", bufs=10, name="pt")
                            nc.scalar.activation(pt[:], ps_s[:], AF.Exp)
                            if 0 <= m:
                                pm = pB.tile([128, TB], f32r, tag="pm", bufs=4, name="pm")
                                nc.vector.tensor_tensor(
                                    pm[:], pt[:], cm_s[:, TB * m:TB * (m + 1)], op=MUL)
                                pt = pm
                            if i == 0:
                                nc.vector.tensor_copy(acc[h][:], pt[:])
                            else:
                                nc.vector.tensor_tensor(acc[h][:], acc[h][:], pt[:], op=ADD)
                            nc.tensor.matmul(ps_o[h][:],
                                             v_all[:, 128 * i:128 * (i + 1)], pt[:],
                                             start=(i == 0), stop=(i == nkt - 1))
                    # normalize into A^T blocks
                    at_s = [pB.tile([128, TB], f32r, tag=f"at{h}", bufs=2, name=f"at_s{h}")
                            for h in range(GH)]
                    for h in range(GH):
                        ps_d = psB.tile([128, TB], f32, tag="tmp", bufs=2, name="ps_d")
                        nc.tensor.matmul(ps_d[:], ones[:], acc[h][:], start=True, stop=True)
                        rec = pB.tile([128, TB], f32, tag="rec", bufs=4, name="rec")
                        nc.vector.reciprocal(rec[:], ps_d[:])
                        nc.vector.tensor_tensor(at_s[h][:], ps_o[h][:], rec[:], op=MUL)

                    # partial o_proj for this q block
                    for co in range(NCT):
                        ps_p = psB.tile([128, TB], f32, tag="tmp", bufs=2, name="ps_p")
                        for jh in range(GH):
                            nc.tensor.matmul(ps_p[:],
                                             wo_all[:, jh * HID + 128 * co:jh * HID + 128 * (co + 1)],
                                             at_s[jh][:], start=(jh == 0), stop=(jh == GH - 1))
                        ob = pB.tile([128, TB], f32, tag="ob", bufs=6, name="ob")
                        nc.vector.tensor_copy(ob[:], ps_p[:])
                        nc.sync.dma_start(
                            oT_part[128 * co:128 * (co + 1), TB * j:TB * (j + 1)], ob[:])

            # ---- phase C: ReduceScatter partials, emit this core's slice ----
            if with_rs:
                nc.gpsimd.collective_compute(
                    "ReduceScatter", ADD,
                    replica_groups=[[0, 1, 2, 3], [4, 5, 6, 7]],
                    ins=[oT_part[:].opt()], outs=[oT_red[:].opt()],
                )
                nc.sync.dma_start(out_r[:], oT_red[:])
            else:
                nc.sync.dma_start(out_r[:], oT_part[0:TB, :])

    nc.compile()
    return nc


def kernel(hidden_states, attention_mask, Wq, Wk, Wv, Wo, sin, cos):
    hidden_states = np.asarray(hidden_states, dtype=np.float32)
    attention_mask = np.asarray(attention_mask, dtype=np.float32)
    Wq, Wk, Wv, Wo = (np.ascontiguousarray(np.asarray(a, dtype=np.float32))
                      for a in (Wq, Wk, Wv, Wo))
    sin = np.asarray(sin, dtype=np.float32)
    cos = np.asarray(cos, dtype=np.float32)

    # classify the mask: causal (top-right strictly very-negative, elsewhere 0,
    # col 0 ignored since reference zeroes it) vs all-zeros (full attention)
    m0 = attention_mask[0, 0]
    iu = np.triu_indices(S, k=1)
    causal = bool((m0[iu] < -1e30).all() and
                  (m0[np.tril_indices(S, k=0)] == 0.0).all())
    if not causal:
        assert (attention_mask == 0).all(), "unsupported attention mask pattern"
    if causal:
        for b in range(1, B):
            assert np.array_equal(attention_mask[b, 0], m0), "mask differs per batch"

    key = causal
    if key not in _CACHE:
        _CACHE[key] = _build(causal)
    nc = _CACHE[key]

    cos_t = np.ascontiguousarray(cos[:S].T)          # [128, S]
    sin_t = cos_t.copy()
    sin_t[:] = sin[:S].T
    sin_m = sin_t.copy()
    sin_m[:64] *= -1.0
    # 0/1 causal keep-patterns for the 4 diagonal alignments
    kl = np.arange(128)[:, None]
    ql = np.arange(TB)[None, :]
    cmask = np.concatenate(
        [(ql >= kl + 128 * m).astype(np.float32) for m in range(4)], axis=1)

    in_maps = []
    for c in range(8):
        b, g = c // 4, c % 4
        in_maps.append({
            "xt": np.ascontiguousarray(hidden_states[b].T),
            "wq": np.ascontiguousarray(Wq[512 * g:512 * (g + 1), :].T),
            "wk": np.ascontiguousarray(Wk[128 * g:128 * (g + 1), :].T),
            "wv": np.ascontiguousarray(Wv[128 * g:128 * (g + 1), :].T),
            "wo": np.ascontiguousarray(Wo[:, 512 * g:512 * (g + 1)].T),
            "cos_t": cos_t, "sin_m": sin_m, "cmask": cmask,
            "ident_in": np.eye(128, dtype=np.float32),
            "ones_in": np.ones((128, 128), dtype=np.float32),
        })

    global _LAST_IN_MAPS
    _LAST_IN_MAPS = in_maps
    res = run_bass_kernel_spmd(nc, in_maps, core_ids=list(range(8)))

    out = np.empty((B, S, HID), dtype=np.float32)
    for c in range(8):
        b, r = c // 4, c % 4
        out[b, :, TB * r:TB * (r + 1)] = res.results[c]["out_r"].T
    return out


if __name__ == "__main__":
    rng = np.random.default_rng(0)
    h = rng.standard_normal((B, S, HID), dtype=np.float32)
    print("module loads ok")



# revision 37
# speedup vs baseline: 1.3177x; 1.3177x over previous
"""HF OpenMoe attention (B=2,S=2048,HID=2048,NH=16,NKV=4,HD=128) on 8 trn2 cores.

Sharding: core c -> (batch b=c//4, kv-group g=c%4). Each core computes Q/K/V
projections for its 4 query heads + 1 kv head, RoPE, causal flash attention in
S^T layout (scores transposed: [k, q], softmax partition-dim reduction via
ones-matmul), and its partial o_proj; a 4-way ReduceScatter sums the o_proj
partials.

Single fused pipeline per 512-token block: project+rope block tb, then
attention for q-block j=tb (its keys are all ready), then partial o_proj.
Causal masking is additive (-32768 via a small bf16 matmul accumulated into
the scores PSUM before exp). Scores for a head pair share one 2-bank PSUM
tile so one Act instruction exponentiates both heads. Softmax denominators
come from a ones-matmul over bf16 exp accumulators. All host-visible tensors
are pre-arranged on the host so every DMA is a plain 2D copy.
"""
import numpy as np
import ml_dtypes
import concourse.bass as bass
import concourse.bacc as bacc
import concourse.tile as tile
import concourse.mybir as mybir
from concourse.bass_utils import run_bass_kernel_spmd

f32 = mybir.dt.float32
f32r = mybir.dt.float32r
bf16 = mybir.dt.bfloat16
AF = mybir.ActivationFunctionType
MUL = mybir.AluOpType.mult
ADD = mybir.AluOpType.add

B, S, HID = 2, 2048, 2048
NH, NKV, HD = 16, 4, 128
GH = NH // NKV          # query heads per core (4)
TB = 512                # token block (q block / projection block)
NT = S // TB            # 4 token blocks
NCT = HID // 128        # 16 contraction tiles
NKT = S // 128          # 16 key tiles

_CACHE = {}
_LAST_IN_MAPS = None


def _build(causal: bool, with_rs: bool = True):
    nc = bacc.Bacc("TRN2", target_bir_lowering=False, debug=False, num_devices=8)
    xt_r = nc.dram_tensor("xt_r", [128, NT * NCT * TB], f32, kind="ExternalInput").ap()
    wq_r = nc.dram_tensor("wq_r", [128, NCT * GH * HD], f32, kind="ExternalInput").ap()
    wk_r = nc.dram_tensor("wk_r", [128, NCT * HD], f32, kind="ExternalInput").ap()
    wv_r = nc.dram_tensor("wv_r", [128, NCT * HD], f32, kind="ExternalInput").ap()
    wo_r = nc.dram_tensor("wo_r", [128, GH * HID], f32, kind="ExternalInput").ap()
    cos_r = nc.dram_tensor("cos_r", [128, S], f32, kind="ExternalInput").ap()
    sin_r = nc.dram_tensor("sin_r", [128, S], f32, kind="ExternalInput").ap()
    cb_r = nc.dram_tensor("cb_r", [128, 512], bf16, kind="ExternalInput").ap()
    u_r = nc.dram_tensor("u_r", [128, 512], bf16, kind="ExternalInput").ap()
    id_r = nc.dram_tensor("id_r", [128, 128], f32, kind="ExternalInput").ap()
    out_r = nc.dram_tensor("out_r", [32, NT * NCT * TB], f32, kind="ExternalOutput").ap()

    with tile.TileContext(nc) as tc:
        with (
            tc.tile_pool(name="glob", bufs=1) as glob,
            tc.tile_pool(name="wk", bufs=1) as wk,
            tc.tile_pool(name="dram", bufs=1, space="DRAM") as dram,
            tc.tile_pool(name="pchain", bufs=2, space="PSUM") as pchain,
            tc.tile_pool(name="pcho", bufs=2, space="PSUM") as pcho,
            tc.tile_pool(name="pscore", bufs=2, space="PSUM") as pscore,
            tc.tile_pool(name="pav", bufs=2, space="PSUM") as pav,
        ):
            # ---- persistent SBUF; DMA order matches block-0 consumption:
            # wq/xt chunks first (Q pass runs first), wk/wv behind, rope
            # tables and mask consts mid-stream, wo chunks after block 0 ----
            wq_all = glob.tile([128, NCT * GH * HD], f32r, tag="wq")  # [c-sub, ci*512+h*128+d]
            wk_all = glob.tile([128, NCT * HD], f32r, tag="wkt")      # [c-sub, ci*128+d]
            wv_all = glob.tile([128, NCT * HD], f32r, tag="wvt")
            cbs = glob.tile([128, 512], bf16, tag="cb")  # [U | negI | ones | -]
            U, negI, onesb = cbs[:, 0:128], cbs[:, 128:256], cbs[:, 256:384]
            U512 = glob.tile([128, 512], bf16, tag="u512")  # full-width mask pattern
            ident = glob.tile([128, 128], f32r, tag="id")
            cos0 = wk.tile([128, TB], f32, tag="cos", bufs=2, name="cos_t")
            sin0 = wk.tile([128, TB], f32, tag="sin", bufs=2, name="sin_t")
            xts0 = []
            for qc in range(4):
                nc.sync.dma_start(wq_all[:, qc * 2048:(qc + 1) * 2048],
                                  wq_r[:, qc * 2048:(qc + 1) * 2048].bitcast(f32r))
                for ci in range(4 * qc, 4 * qc + 4):
                    t0 = wk.tile([128, TB], f32r, tag="xt", bufs=18, name="xt")
                    nc.sync.dma_start(t0[:], xt_r[:, ci * TB:(ci + 1) * TB].bitcast(f32r))
                    xts0.append(t0)
                sl = slice(qc * 512, (qc + 1) * 512)
                nc.sync.dma_start(wk_all[:, sl], wk_r[:, sl].bitcast(f32r))
                nc.sync.dma_start(wv_all[:, sl], wv_r[:, sl].bitcast(f32r))
                if qc == 1:
                    nc.sync.dma_start(cos0[:], cos_r[:, 0:TB])
                    nc.sync.dma_start(sin0[:], sin_r[:, 0:TB])
                if qc == 2:
                    nc.sync.dma_start(cbs[:], cb_r[:])
                    nc.sync.dma_start(U512[:], u_r[:])
                    nc.sync.dma_start(ident[:], id_r[:].bitcast(f32r))
            wo_all = glob.tile([128, GH * HID], f32r, tag="wo")  # [d-sub, co*512+jh*128+c]
            for qc in range(4):
                nc.sync.dma_start(wo_all[:, qc * 2048:(qc + 1) * 2048],
                                  wo_r[:, qc * 2048:(qc + 1) * 2048].bitcast(f32r))

            kt_rope = glob.tile([128, S], f32r, tag="kt")             # roped K^T [d, k]
            v_all = glob.tile([128, S], bf16, tag="v")                # V natural, tile i at 128i

            if causal:
                qt = None  # per-block work tiles
            else:
                qt = [glob.tile([128, S], f32r, tag=f"qtg{h}", name=f"qt_g{h}")
                      for h in range(GH)]

            oT_part = dram.tile([128, NT * NCT * TB], f32)            # o^T partial (rearranged)
            oT_red = dram.tile([32, NT * NCT * TB], f32)

            def rope(ps, dst_ap, cos_t, sin_t):
                """dst = ps*cos + rot(ps)*sin_m (rotate_half sign in sin_m)."""
                raw = wk.tile([128, TB], f32, tag="raw", bufs=3, name="raw")
                nc.scalar.copy(raw[:], ps[:])
                rot = wk.tile([128, TB], f32, tag="rot", bufs=2, name="rot")
                nc.sync.dma_start(rot[0:64, :], raw[64:128, :])
                nc.sync.dma_start(rot[64:128, :], raw[0:64, :])
                m1 = wk.tile([128, TB], f32, tag="m1", bufs=2, name="m1")
                nc.vector.tensor_tensor(m1[:], raw[:], cos_t, op=MUL)
                m2 = wk.tile([128, TB], f32, tag="m2", bufs=2, name="m2")
                nc.vector.tensor_tensor(m2[:], rot[:], sin_t, op=MUL)
                nc.vector.tensor_tensor(dst_ap, m1[:], m2[:], op=ADD)

            def attention(j, qt_j):
                """Causal-aware attention for q block j + partial o_proj."""
                nkt = 4 * (j + 1) if causal else NKT
                at_s = []
                for pair in range(2):
                    heads = (2 * pair, 2 * pair + 1)
                    ps_o = {h: pav.tile([128, TB], f32, tag="po", name=f"ps_o{h}")
                            for h in heads}
                    acc = {h: wk.tile([128, TB], bf16, tag="acc", bufs=2,
                                      name=f"acc{h}") for h in heads}
                    for i in range(nkt):
                        m = i - 4 * j if causal else -1
                        a = 128 * m if m > 0 else 0          # valid q cols [a, TB)
                        for h in heads:
                            qsl = qt_j[h] if a == 0 else qt_j[h][:, a:TB]
                            ps_s = pscore.tile([128, TB], f32, tag="ps", name="ps_s")
                            if m >= 0:
                                # additive causal mask goes FIRST with the
                                # start (=zeroing) bit: its operands are
                                # constants so it is always ready before the
                                # scores matmul below — the scheduler can
                                # never reorder the zeroing write after it.
                                nc.tensor.matmul(ps_s[:, a:TB], negI,
                                                 U512[:, 0:TB - a],
                                                 start=True, stop=False)
                            nc.tensor.matmul(ps_s[:, a:TB],
                                             kt_rope[:, 128 * i:128 * (i + 1)],
                                             qsl, start=(m < 0), stop=True)
                            if i == 0:
                                pt = acc[h]
                                nc.scalar.activation(pt[:, a:TB], ps_s[:, a:TB], AF.Exp)
                            else:
                                pt = wk.tile([128, TB], bf16, tag="pt", bufs=6, name="pt")
                                nc.scalar.activation(pt[:, a:TB], ps_s[:, a:TB], AF.Exp)
                                nc.vector.tensor_tensor(acc[h][:, a:TB], acc[h][:, a:TB],
                                                        pt[:, a:TB], op=ADD)
                            nc.tensor.matmul(ps_o[h][:, a:TB],
                                             v_all[:, 128 * i:128 * (i + 1)], pt[:, a:TB],
                                             start=(i == 0), stop=(i == nkt - 1))
                    for h in heads:
                        ps_d = pcho.tile([128, TB], f32, tag="cho", name="ps_d")
                        nc.tensor.matmul(ps_d[:], onesb, acc[h][:], start=True, stop=True)
                        rec = wk.tile([128, TB], f32, tag="rec", bufs=2, name="rec")
                        nc.vector.reciprocal(rec[:], ps_d[:])
                        at = wk.tile([128, TB], f32r, tag=f"at{h}", bufs=1, name=f"at{h}")
                        nc.vector.tensor_tensor(at[:], ps_o[h][:], rec[:], op=MUL)
                        at_s.append(at)

                # partial o_proj for q block j: 16 co chains of 4 matmuls
                for cg in range(8):
                    ob = wk.tile([128, 1024], f32, tag="ob", bufs=2, name="ob")
                    for u in range(2):
                        co = 2 * cg + u
                        ps_p = pcho.tile([128, TB], f32, tag="cho", name="ps_p")
                        for jh in range(GH):
                            nc.tensor.matmul(
                                ps_p[:],
                                wo_all[:, co * 512 + 128 * jh:co * 512 + 128 * (jh + 1)],
                                at_s[jh][:], start=(jh == 0), stop=(jh == GH - 1))
                        if u == 0:
                            nc.scalar.copy(ob[:, 0:512], ps_p[:])
                        else:
                            nc.vector.tensor_copy(ob[:, 512:1024], ps_p[:])
                    sl = slice(j * 8192 + cg * 1024, j * 8192 + (cg + 1) * 1024)
                    nc.sync.dma_start(oT_part[:, sl], ob[:])
                    if not with_rs:
                        # emit this core's output slice incrementally (the RS
                        # build instead reduces oT_part across the group)
                        nc.sync.dma_start(out_r[:, sl], ob[0:32, :])

            # ---- fused per-block pipeline ----
            for tb in range(NT):
                if tb == 0:
                    xts, cos_t, sin_t = xts0, cos0, sin0
                else:
                    xts = []
                    for ci in range(NCT):
                        t = wk.tile([128, TB], f32r, tag="xt", bufs=18, name="xt")
                        # issued via the idle Pool engine's SWDGE so these
                        # don't queue behind the weight loads on SP; the xt
                        # buffer rotation paces the transfers naturally
                        nc.gpsimd.dma_start(
                            t[:], xt_r[:, (tb * NCT + ci) * TB:(tb * NCT + ci + 1) * TB]
                            .bitcast(f32r))
                        xts.append(t)
                    cos_t = wk.tile([128, TB], f32, tag="cos", bufs=2, name="cos_t")
                    nc.sync.dma_start(cos_t[:], cos_r[:, TB * tb:TB * (tb + 1)])
                    sin_t = wk.tile([128, TB], f32, tag="sin", bufs=2, name="sin_t")
                    nc.sync.dma_start(sin_t[:], sin_r[:, TB * tb:TB * (tb + 1)])

                # passes 1+2: Q projection chains (pairs), rope into qt tiles.
                # Q first: attention's off-diagonal tiles only need this
                # block's Q (K/V come from earlier blocks), so it can start
                # while the K/V pass is still running.
                if causal:
                    qt_j = [wk.tile([128, TB], f32r, tag=f"qt{h}", bufs=2, name=f"qt{h}")
                            for h in range(GH)]
                else:
                    qt_j = None
                for qp in range(2):
                    ps_q = {}
                    for h in (2 * qp, 2 * qp + 1):
                        ps_q[h] = pchain.tile([128, TB], f32, tag="chain", name=f"ps_q{h}")
                    for ci in range(NCT):
                        st, sp = ci == 0, ci == NCT - 1
                        for h in (2 * qp, 2 * qp + 1):
                            nc.tensor.matmul(
                                ps_q[h][:],
                                wq_all[:, ci * 512 + 128 * h:ci * 512 + 128 * (h + 1)],
                                xts[ci][:], start=st, stop=sp)
                    for h in (2 * qp, 2 * qp + 1):
                        dst = qt_j[h][:] if causal else qt[h][:, TB * tb:TB * (tb + 1)]
                        rope(ps_q[h], dst, cos_t, sin_t)

                # pass 3: K and V projection chains
                ps_k = pchain.tile([128, TB], f32, tag="chain", name="ps_k")
                ps_v = pchain.tile([128, TB], f32, tag="chain", name="ps_v")
                for ci in range(NCT):
                    st, sp = ci == 0, ci == NCT - 1
                    nc.tensor.matmul(ps_k[:], wk_all[:, 128 * ci:128 * (ci + 1)],
                                     xts[ci][:], start=st, stop=sp)
                    nc.tensor.matmul(ps_v[:], wv_all[:, 128 * ci:128 * (ci + 1)],
                                     xts[ci][:], start=st, stop=sp)
                rope(ps_k, kt_rope[:, TB * tb:TB * (tb + 1)], cos_t, sin_t)
                vt_sb = wk.tile([128, TB], f32r, tag="vts", bufs=2, name="vt_sb")
                nc.vector.tensor_copy(vt_sb[:], ps_v[:])
                for u in range(TB // 128):
                    ps_tr = pchain.tile([128, 128], f32r, tag="chain", name="ps_tr")
                    nc.tensor.transpose(ps_tr[:], vt_sb[:, 128 * u:128 * (u + 1)],
                                        ident[:])
                    nc.scalar.copy(
                        v_all[:, 128 * (4 * tb + u):128 * (4 * tb + u + 1)],
                        ps_tr[:].bitcast(f32))

                if causal:
                    attention(tb, qt_j)

            if not causal:
                for j in range(NT):
                    attention(j, [q[:, TB * j:TB * (j + 1)] for q in qt])

            # ---- ReduceScatter partials, emit this core's slice ----
            if with_rs:
                nc.gpsimd.collective_compute(
                    "ReduceScatter", ADD,
                    replica_groups=[[0, 1, 2, 3], [4, 5, 6, 7]],
                    ins=[oT_part[:].opt()], outs=[oT_red[:].opt()],
                )
                nc.sync.dma_start(out_r[:], oT_red[:])

    nc.compile()
    return nc


def _host_arrange(a2d, nblk):
    """[nblk*128, F] -> [128, nblk*F] with block-major free dim."""
    n, fdim = a2d.shape
    assert n == nblk * 128
    return np.ascontiguousarray(
        a2d.reshape(nblk, 128, fdim).transpose(1, 0, 2).reshape(128, nblk * fdim))


def kernel(hidden_states, attention_mask, Wq, Wk, Wv, Wo, sin, cos):
    hidden_states = np.asarray(hidden_states, dtype=np.float32)
    attention_mask = np.asarray(attention_mask, dtype=np.float32)
    Wq, Wk, Wv, Wo = (np.ascontiguousarray(np.asarray(a, dtype=np.float32))
                      for a in (Wq, Wk, Wv, Wo))
    sin = np.asarray(sin, dtype=np.float32)
    cos = np.asarray(cos, dtype=np.float32)

    # classify the mask: causal (top-right strictly very-negative, elsewhere 0,
    # col 0 ignored since reference zeroes it) vs all-zeros (full attention)
    m0 = attention_mask[0, 0]
    iu = np.triu_indices(S, k=1)
    causal = bool((m0[iu] < -1e30).all() and
                  (m0[np.tril_indices(S, k=0)] == 0.0).all())
    if not causal:
        assert (attention_mask == 0).all(), "unsupported attention mask pattern"
    if causal:
        for b in range(1, B):
            assert np.array_equal(attention_mask[b, 0], m0), "mask differs per batch"

    key = causal
    if key not in _CACHE:
        _CACHE[key] = _build(causal)
    nc = _CACHE[key]

    cos_t = np.ascontiguousarray(cos[:S].T)          # [128, S]
    sin_m = np.ascontiguousarray(sin[:S].T)
    sin_m[:64] *= -1.0                               # rotate_half sign
    # bf16 const block: [U | negI | ones | unused]
    Um = (np.arange(512)[None, :] < np.arange(128)[:, None]).astype(np.float32)
    Um[:, 128:] = 0.0
    negI = np.eye(128, dtype=np.float32) * -32768.0
    onesb = np.ones((128, 128), dtype=np.float32)
    identb = np.eye(128, dtype=np.float32)
    cb = np.concatenate([Um[:, :128], negI, onesb, np.zeros((128, 128), np.float32)],
                        axis=1).astype(ml_dtypes.bfloat16)
    u512 = Um.astype(ml_dtypes.bfloat16)

    in_maps = []
    for c in range(8):
        b, g = c // 4, c % 4
        xt = np.ascontiguousarray(hidden_states[b].T)          # [HID, S]
        # [128, tb*16*512 + ci*512 + col] <- xt[128*ci+p, 512*tb+col]
        xt_r = np.ascontiguousarray(
            xt.reshape(NCT, 128, NT, TB).transpose(1, 2, 0, 3).reshape(128, NT * NCT * TB))
        wq_r = _host_arrange(Wq[512 * g:512 * (g + 1), :].T, NCT)
        wk_r = _host_arrange(Wk[128 * g:128 * (g + 1), :].T, NCT)
        wv_r = _host_arrange(Wv[128 * g:128 * (g + 1), :].T, NCT)
        # co-major o_proj weights: wo_r[p, co*512+jh*128+cc] = Wo[128co+cc, 512g+128jh+p]
        wo_r = np.ascontiguousarray(
            Wo[:, 512 * g:512 * (g + 1)].reshape(NCT, 128, GH, 128)
            .transpose(3, 0, 2, 1).reshape(128, GH * HID))
        in_maps.append({
            "xt_r": xt_r, "wq_r": wq_r, "wk_r": wk_r, "wv_r": wv_r, "wo_r": wo_r,
            "cos_r": cos_t, "sin_r": sin_m, "cb_r": cb, "u_r": u512, "id_r": identb,
        })

    global _LAST_IN_MAPS
    _LAST_IN_MAPS = in_maps
    res = run_bass_kernel_spmd(nc, in_maps, core_ids=list(range(8)))

    out = np.empty((B, S, HID), dtype=np.float32)
    obuf = np.empty((NCT, 128, NT, TB), dtype=np.float32)  # [co, p, j, col]
    for b in range(B):
        for r in range(4):
            part = res.results[4 * b + r]["out_r"]          # [32, j*8192+co*512+col]
            obuf[:, 32 * r:32 * (r + 1)] = (
                part.reshape(32, NT, NCT, TB).transpose(2, 0, 1, 3))
        out[b] = obuf.reshape(HID, S).T
    return out


if __name__ == "__main__":
    print("module loads ok")


# revision 60
# speedup vs baseline: 1.3192x; 1.0012x over previous
"""HF OpenMoe attention (B=2,S=2048,HID=2048,NH=16,NKV=4,HD=128) on 8 trn2 cores.

Sharding: core c -> (batch b=c//4, kv-group g=c%4). Each core computes Q/K/V
projections for its 4 query heads + 1 kv head, RoPE, causal flash attention in
S^T layout (scores transposed: [k, q], softmax partition-dim reduction via
ones-matmul), and its partial o_proj; a 4-way ReduceScatter sums the o_proj
partials.

Single fused pipeline per 512-token block: project+rope block tb, then
attention for q-block j=tb (its keys are all ready), then partial o_proj.
Causal masking is additive (-32768 via a small bf16 matmul accumulated into
the scores PSUM before exp). Scores for a head pair share one 2-bank PSUM
tile so one Act instruction exponentiates both heads. Softmax denominators
come from a ones-matmul over bf16 exp accumulators. All host-visible tensors
are pre-arranged on the host so every DMA is a plain 2D copy.
"""
import numpy as np
import ml_dtypes
import concourse.bass as bass
import concourse.bacc as bacc
import concourse.tile as tile
import concourse.mybir as mybir
from concourse.bass_utils import run_bass_kernel_spmd

f32 = mybir.dt.float32
f32r = mybir.dt.float32r
bf16 = mybir.dt.bfloat16
AF = mybir.ActivationFunctionType
MUL = mybir.AluOpType.mult
ADD = mybir.AluOpType.add

B, S, HID = 2, 2048, 2048
NH, NKV, HD = 16, 4, 128
GH = NH // NKV          # query heads per core (4)
TB = 512                # token block (q block / projection block)
NT = S // TB            # 4 token blocks
NCT = HID // 128        # 16 contraction tiles
NKT = S // 128          # 16 key tiles

_CACHE = {}
_LAST_IN_MAPS = None


def _build(causal: bool, with_rs: bool = True):
    nc = bacc.Bacc("TRN2", target_bir_lowering=False, debug=False, num_devices=8)
    xt_r = nc.dram_tensor("xt_r", [128, NT * NCT * TB], f32, kind="ExternalInput").ap()
    wq_r = nc.dram_tensor("wq_r", [128, NCT * GH * HD], f32, kind="ExternalInput").ap()
    wk_r = nc.dram_tensor("wk_r", [128, NCT * HD], f32, kind="ExternalInput").ap()
    wv_r = nc.dram_tensor("wv_r", [128, NCT * HD], f32, kind="ExternalInput").ap()
    wo_r = nc.dram_tensor("wo_r", [128, GH * HID], f32, kind="ExternalInput").ap()
    cos_r = nc.dram_tensor("cos_r", [128, S], f32, kind="ExternalInput").ap()
    sin_r = nc.dram_tensor("sin_r", [128, S], f32, kind="ExternalInput").ap()
    cb_r = nc.dram_tensor("cb_r", [128, 512], bf16, kind="ExternalInput").ap()
    u_r = nc.dram_tensor("u_r", [128, 512], bf16, kind="ExternalInput").ap()
    id_r = nc.dram_tensor("id_r", [128, 128], f32, kind="ExternalInput").ap()
    out_r = nc.dram_tensor("out_r", [32, NT * NCT * TB], f32, kind="ExternalOutput").ap()

    with tile.TileContext(nc) as tc:
        with (
            tc.tile_pool(name="glob", bufs=1) as glob,
            tc.tile_pool(name="wk", bufs=1) as wk,
            tc.tile_pool(name="dram", bufs=1, space="DRAM") as dram,
            tc.tile_pool(name="pchain", bufs=2, space="PSUM") as pchain,
            tc.tile_pool(name="pcho", bufs=2, space="PSUM") as pcho,
            tc.tile_pool(name="pscore", bufs=2, space="PSUM") as pscore,
            tc.tile_pool(name="pav", bufs=2, space="PSUM") as pav,
        ):
            # ---- persistent SBUF; DMA order matches block-0 consumption:
            # wq/xt chunks first (Q pass runs first), wk/wv behind, rope
            # tables and mask consts mid-stream, wo chunks after block 0 ----
            wq_all = glob.tile([128, NCT * GH * HD], f32r, tag="wq")  # [c-sub, ci*512+h*128+d]
            wk_all = glob.tile([128, NCT * HD], f32r, tag="wkt")      # [c-sub, ci*128+d]
            wv_all = glob.tile([128, NCT * HD], f32r, tag="wvt")
            cbs = glob.tile([128, 512], bf16, tag="cb")  # [U | negI | ones | -]
            U, negI, onesb = cbs[:, 0:128], cbs[:, 128:256], cbs[:, 256:384]
            U512 = glob.tile([128, 512], bf16, tag="u512")  # full-width mask pattern
            ident = glob.tile([128, 128], f32r, tag="id")
            cos0 = wk.tile([128, TB], f32, tag="cos", bufs=2, name="cos_t")
            sin0 = wk.tile([128, TB], f32, tag="sin", bufs=2, name="sin_t")
            xts0 = []
            for qc in range(4):
                nc.sync.dma_start(wq_all[:, qc * 2048:(qc + 1) * 2048],
                                  wq_r[:, qc * 2048:(qc + 1) * 2048].bitcast(f32r))
                for ci in range(4 * qc, 4 * qc + 4):
                    t0 = wk.tile([128, TB], f32r, tag="xt", bufs=18, name="xt")
                    nc.sync.dma_start(t0[:], xt_r[:, ci * TB:(ci + 1) * TB].bitcast(f32r))
                    xts0.append(t0)
                sl = slice(qc * 512, (qc + 1) * 512)
                nc.sync.dma_start(wk_all[:, sl], wk_r[:, sl].bitcast(f32r))
                nc.sync.dma_start(wv_all[:, sl], wv_r[:, sl].bitcast(f32r))
                if qc == 1:
                    nc.sync.dma_start(cos0[:], cos_r[:, 0:TB])
                    nc.sync.dma_start(sin0[:], sin_r[:, 0:TB])
                if qc == 2:
                    nc.sync.dma_start(cbs[:], cb_r[:])
                    nc.sync.dma_start(U512[:], u_r[:])
                    nc.sync.dma_start(ident[:], id_r[:].bitcast(f32r))
            wo_all = glob.tile([128, GH * HID], f32r, tag="wo")  # [d-sub, co*512+jh*128+c]
            for qc in range(4):
                nc.sync.dma_start(wo_all[:, qc * 2048:(qc + 1) * 2048],
                                  wo_r[:, qc * 2048:(qc + 1) * 2048].bitcast(f32r))

            kt_rope = glob.tile([128, S], f32r, tag="kt")             # roped K^T [d, k]
            v_all = glob.tile([128, S], bf16, tag="v")                # V natural, tile i at 128i

            if causal:
                qt = None  # per-block work tiles
            else:
                qt = [glob.tile([128, S], f32r, tag=f"qtg{h}", name=f"qt_g{h}")
                      for h in range(GH)]

            oT_part = dram.tile([128, NT * NCT * TB], f32)            # o^T partial (rearranged)
            oT_red = dram.tile([32, NT * NCT * TB], f32)

            def rope(ps, dst_ap, cos_t, sin_t):
                """dst = ps*cos + rot(ps)*sin_m (rotate_half sign in sin_m)."""
                raw = wk.tile([128, TB], f32, tag="raw", bufs=3, name="raw")
                nc.scalar.copy(raw[:], ps[:])
                rot = wk.tile([128, TB], f32, tag="rot", bufs=2, name="rot")
                nc.sync.dma_start(rot[0:64, :], raw[64:128, :])
                nc.sync.dma_start(rot[64:128, :], raw[0:64, :])
                m1 = wk.tile([128, TB], f32, tag="m1", bufs=2, name="m1")
                nc.vector.tensor_tensor(m1[:], raw[:], cos_t, op=MUL)
                m2 = wk.tile([128, TB], f32, tag="m2", bufs=2, name="m2")
                nc.vector.tensor_tensor(m2[:], rot[:], sin_t, op=MUL)
                nc.vector.tensor_tensor(dst_ap, m1[:], m2[:], op=ADD)

            def attention(j, qt_j):
                """Causal-aware attention for q block j + partial o_proj.
                Heads are processed singly: with po double-buffered, head
                h+1's AV accumulation starts while head h normalizes, so
                there is no inter-head pipeline bubble."""
                nkt = 4 * (j + 1) if causal else NKT
                at_s = []
                for h in range(GH):
                    ps_o = pav.tile([128, TB], f32, tag="po", name=f"ps_o{h}")
                    acc = wk.tile([128, TB], bf16, tag="acc", bufs=2, name=f"acc{h}")
                    for i in range(nkt):
                        m = i - 4 * j if causal else -1
                        a = 128 * m if m > 0 else 0          # valid q cols [a, TB)
                        qsl = qt_j[h] if a == 0 else qt_j[h][:, a:TB]
                        ps_s = pscore.tile([128, TB], f32, tag="ps", name="ps_s")
                        if m >= 0:
                            # additive causal mask goes FIRST with the start
                            # (=zeroing) bit: its operands are constants so it
                            # is always ready before the scores matmul below —
                            # the scheduler can never reorder the zeroing
                            # write after it.
                            nc.tensor.matmul(ps_s[:, a:TB], negI,
                                             U512[:, 0:TB - a],
                                             start=True, stop=False)
                        nc.tensor.matmul(ps_s[:, a:TB],
                                         kt_rope[:, 128 * i:128 * (i + 1)],
                                         qsl, start=(m < 0), stop=True)
                        if i == 0:
                            pt = acc
                            nc.scalar.activation(pt[:, a:TB], ps_s[:, a:TB], AF.Exp)
                        else:
                            pt = wk.tile([128, TB], bf16, tag="pt", bufs=6, name="pt")
                            nc.scalar.activation(pt[:, a:TB], ps_s[:, a:TB], AF.Exp)
                            nc.vector.tensor_tensor(acc[:, a:TB], acc[:, a:TB],
                                                    pt[:, a:TB], op=ADD)
                        nc.tensor.matmul(ps_o[:, a:TB],
                                         v_all[:, 128 * i:128 * (i + 1)], pt[:, a:TB],
                                         start=(i == 0), stop=(i == nkt - 1))
                    ps_d = pcho.tile([128, TB], f32, tag="cho", name="ps_d")
                    nc.tensor.matmul(ps_d[:], onesb, acc[:], start=True, stop=True)
                    rec = wk.tile([128, TB], f32, tag="rec", bufs=2, name="rec")
                    nc.vector.reciprocal(rec[:], ps_d[:])
                    at = wk.tile([128, TB], f32r, tag=f"at{h}", bufs=1, name=f"at{h}")
                    nc.vector.tensor_tensor(at[:], ps_o[:], rec[:], op=MUL)
                    at_s.append(at)

                # partial o_proj for q block j: 16 co chains of 4 matmuls
                for cg in range(8):
                    ob = wk.tile([128, 1024], f32, tag="ob", bufs=2, name="ob")
                    for u in range(2):
                        co = 2 * cg + u
                        ps_p = pcho.tile([128, TB], f32, tag="cho", name="ps_p")
                        for jh in range(GH):
                            nc.tensor.matmul(
                                ps_p[:],
                                wo_all[:, co * 512 + 128 * jh:co * 512 + 128 * (jh + 1)],
                                at_s[jh][:], start=(jh == 0), stop=(jh == GH - 1))
                        if u == 0:
                            nc.scalar.copy(ob[:, 0:512], ps_p[:])
                        else:
                            nc.vector.tensor_copy(ob[:, 512:1024], ps_p[:])
                    sl = slice(j * 8192 + cg * 1024, j * 8192 + (cg + 1) * 1024)
                    nc.sync.dma_start(oT_part[:, sl], ob[:])
                    if not with_rs:
                        # emit this core's output slice incrementally (the RS
                        # build instead reduces oT_part across the group)
                        nc.sync.dma_start(out_r[:, sl], ob[0:32, :])

            # ---- fused per-block pipeline ----
            for tb in range(NT):
                if tb == 0:
                    xts, cos_t, sin_t = xts0, cos0, sin0
                else:
                    xts = []
                    for ci in range(NCT):
                        t = wk.tile([128, TB], f32r, tag="xt", bufs=18, name="xt")
                        # issued via the idle Pool engine's SWDGE so these
                        # don't queue behind the weight loads on SP; the xt
                        # buffer rotation paces the transfers naturally
                        nc.gpsimd.dma_start(
                            t[:], xt_r[:, (tb * NCT + ci) * TB:(tb * NCT + ci + 1) * TB]
                            .bitcast(f32r))
                        xts.append(t)
                    cos_t = wk.tile([128, TB], f32, tag="cos", bufs=2, name="cos_t")
                    nc.sync.dma_start(cos_t[:], cos_r[:, TB * tb:TB * (tb + 1)])
                    sin_t = wk.tile([128, TB], f32, tag="sin", bufs=2, name="sin_t")
                    nc.sync.dma_start(sin_t[:], sin_r[:, TB * tb:TB * (tb + 1)])

                # passes 1+2: Q projection chains (pairs), rope into qt tiles.
                # Q first: attention's off-diagonal tiles only need this
                # block's Q (K/V come from earlier blocks), so it can start
                # while the K/V pass is still running.
                if causal:
                    qt_j = [wk.tile([128, TB], f32r, tag=f"qt{h}", bufs=2, name=f"qt{h}")
                            for h in range(GH)]
                else:
                    qt_j = None
                for qp in range(2):
                    ps_q = {}
                    for h in (2 * qp, 2 * qp + 1):
                        ps_q[h] = pchain.tile([128, TB], f32, tag="chain", name=f"ps_q{h}")
                    for ci in range(NCT):
                        st, sp = ci == 0, ci == NCT - 1
                        for h in (2 * qp, 2 * qp + 1):
                            nc.tensor.matmul(
                                ps_q[h][:],
                                wq_all[:, ci * 512 + 128 * h:ci * 512 + 128 * (h + 1)],
                                xts[ci][:], start=st, stop=sp)
                    for h in (2 * qp, 2 * qp + 1):
                        dst = qt_j[h][:] if causal else qt[h][:, TB * tb:TB * (tb + 1)]
                        rope(ps_q[h], dst, cos_t, sin_t)

                # pass 3: K and V projection chains
                ps_k = pchain.tile([128, TB], f32, tag="chain", name="ps_k")
                ps_v = pchain.tile([128, TB], f32, tag="chain", name="ps_v")
                for ci in range(NCT):
                    st, sp = ci == 0, ci == NCT - 1
                    nc.tensor.matmul(ps_k[:], wk_all[:, 128 * ci:128 * (ci + 1)],
                                     xts[ci][:], start=st, stop=sp)
                    nc.tensor.matmul(ps_v[:], wv_all[:, 128 * ci:128 * (ci + 1)],
                                     xts[ci][:], start=st, stop=sp)
                rope(ps_k, kt_rope[:, TB * tb:TB * (tb + 1)], cos_t, sin_t)
                vt_sb = wk.tile([128, TB], f32r, tag="vts", bufs=2, name="vt_sb")
                nc.vector.tensor_copy(vt_sb[:], ps_v[:])
                for u in range(TB // 128):
                    ps_tr = pchain.tile([128, 128], f32r, tag="chain", name="ps_tr")
                    nc.tensor.transpose(ps_tr[:], vt_sb[:, 128 * u:128 * (u + 1)],
                                        ident[:])
                    nc.scalar.copy(
                        v_all[:, 128 * (4 * tb + u):128 * (4 * tb + u + 1)],
                        ps_tr[:].bitcast(f32))

                if causal:
                    attention(tb, qt_j)

            if not causal:
                for j in range(NT):
                    attention(j, [q[:, TB * j:TB * (j + 1)] for q in qt])

            # ---- ReduceScatter partials, emit this core's slice ----
            if with_rs:
                nc.gpsimd.collective_compute(
                    "ReduceScatter", ADD,
                    replica_groups=[[0, 1, 2, 3], [4, 5, 6, 7]],
                    ins=[oT_part[:].opt()], outs=[oT_red[:].opt()],
                )
                nc.sync.dma_start(out_r[:], oT_red[:])

    nc.compile()
    return nc


def _host_arrange(a2d, nblk):
    """[nblk*128, F] -> [128, nblk*F] with block-major free dim."""
    n, fdim = a2d.shape
    assert n == nblk * 128
    return np.ascontiguousarray(
        a2d.reshape(nblk, 128, fdim).transpose(1, 0, 2).reshape(128, nblk * fdim))


def kernel(hidden_states, attention_mask, Wq, Wk, Wv, Wo, sin, cos):
    hidden_states = np.asarray(hidden_states, dtype=np.float32)
    attention_mask = np.asarray(attention_mask, dtype=np.float32)
    Wq, Wk, Wv, Wo = (np.ascontiguousarray(np.asarray(a, dtype=np.float32))
                      for a in (Wq, Wk, Wv, Wo))
    sin = np.asarray(sin, dtype=np.float32)
    cos = np.asarray(cos, dtype=np.float32)

    # classify the mask: causal (top-right strictly very-negative, elsewhere 0,
    # col 0 ignored since reference zeroes it) vs all-zeros (full attention)
    m0 = attention_mask[0, 0]
    iu = np.triu_indices(S, k=1)
    causal = bool((m0[iu] < -1e30).all() and
                  (m0[np.tril_indices(S, k=0)] == 0.0).all())
    if not causal:
        assert (attention_mask == 0).all(), "unsupported attention mask pattern"
    if causal:
        for b in range(1, B):
            assert np.array_equal(attention_mask[b, 0], m0), "mask differs per batch"

    key = causal
    if key not in _CACHE:
        _CACHE[key] = _build(causal)
    nc = _CACHE[key]

    cos_t = np.ascontiguousarray(cos[:S].T)          # [128, S]
    sin_m = np.ascontiguousarray(sin[:S].T)
    sin_m[:64] *= -1.0                               # rotate_half sign
    # bf16 const block: [U | negI | ones | unused]
    Um = (np.arange(512)[None, :] < np.arange(128)[:, None]).astype(np.float32)
    Um[:, 128:] = 0.0
    negI = np.eye(128, dtype=np.float32) * -32768.0
    onesb = np.ones((128, 128), dtype=np.float32)
    identb = np.eye(128, dtype=np.float32)
    cb = np.concatenate([Um[:, :128], negI, onesb, np.zeros((128, 128), np.float32)],
                        axis=1).astype(ml_dtypes.bfloat16)
    u512 = Um.astype(ml_dtypes.bfloat16)

    in_maps = []
    for c in range(8):
        b, g = c // 4, c % 4
        xt = np.ascontiguousarray(hidden_states[b].T)          # [HID, S]
        # [128, tb*16*512 + ci*512 + col] <- xt[128*ci+p, 512*tb+col]
        xt_r = np.ascontiguousarray(
            xt.reshape(NCT, 128, NT, TB).transpose(1, 2, 0, 3).reshape(128, NT * NCT * TB))
        wq_r = _host_arrange(Wq[512 * g:512 * (g + 1), :].T, NCT)
        wk_r = _host_arrange(Wk[128 * g:128 * (g + 1), :].T, NCT)
        wv_r = _host_arrange(Wv[128 * g:128 * (g + 1), :].T, NCT)
        # co-major o_proj weights: wo_r[p, co*512+jh*128+cc] = Wo[128co+cc, 512g+128jh+p]
        wo_r = np.ascontiguousarray(
            Wo[:, 512 * g:512 * (g + 1)].reshape(NCT, 128, GH, 128)
            .transpose(3, 0, 2, 1).reshape(128, GH * HID))
        in_maps.append({
            "xt_r": xt_r, "wq_r": wq_r, "wk_r": wk_r, "wv_r": wv_r, "wo_r": wo_r,
            "cos_r": cos_t, "sin_r": sin_m, "cb_r": cb, "u_r": u512, "id_r": identb,
        })

    global _LAST_IN_MAPS
    _LAST_IN_MAPS = in_maps
    res = run_bass_kernel_spmd(nc, in_maps, core_ids=list(range(8)))

    out = np.empty((B, S, HID), dtype=np.float32)
    obuf = np.empty((NCT, 128, NT, TB), dtype=np.float32)  # [co, p, j, col]
    for b in range(B):
        for r in range(4):
            part = res.results[4 * b + r]["out_r"]          # [32, j*8192+co*512+col]
            obuf[:, 32 * r:32 * (r + 1)] = (
                part.reshape(32, NT, NCT, TB).transpose(2, 0, 1, 3))
        out[b] = obuf.reshape(HID, S).T
    return out


if __name__ == "__main__":
    print("module loads ok")


# revision 71
# speedup vs baseline: 1.3453x; 1.0197x over previous
"""HF OpenMoe attention (B=2,S=2048,HID=2048,NH=16,NKV=4,HD=128) on 8 trn2 cores.

Sharding: core c -> (batch b=c//4, kv-group g=c%4). Each core computes Q/K/V
projections for its 4 query heads + 1 kv head, RoPE, causal flash attention in
S^T layout (scores transposed: [k, q], softmax partition-dim reduction via
ones-matmul), and its partial o_proj; a 4-way ReduceScatter sums the o_proj
partials.

Single fused pipeline per 512-token block: project+rope block tb, then
attention for q-block j=tb (its keys are all ready), then partial o_proj.
Causal masking is additive (-32768 via a small bf16 matmul accumulated into
the scores PSUM before exp). Scores for a head pair share one 2-bank PSUM
tile so one Act instruction exponentiates both heads. Softmax denominators
come from a ones-matmul over bf16 exp accumulators. All host-visible tensors
are pre-arranged on the host so every DMA is a plain 2D copy.
"""
import numpy as np
import ml_dtypes
import concourse.bass as bass
import concourse.bacc as bacc
import concourse.tile as tile
import concourse.mybir as mybir
from concourse.bass_utils import run_bass_kernel_spmd

f32 = mybir.dt.float32
f32r = mybir.dt.float32r
bf16 = mybir.dt.bfloat16
AF = mybir.ActivationFunctionType
MUL = mybir.AluOpType.mult
ADD = mybir.AluOpType.add

B, S, HID = 2, 2048, 2048
NH, NKV, HD = 16, 4, 128
GH = NH // NKV          # query heads per core (4)
TB = 512                # token block (q block / projection block)
NT = S // TB            # 4 token blocks
NCT = HID // 128        # 16 contraction tiles
NKT = S // 128          # 16 key tiles

_CACHE = {}
_LAST_IN_MAPS = None


def _build(causal: bool, with_rs: bool = True):
    nc = bacc.Bacc("TRN2", target_bir_lowering=False, debug=False, num_devices=8)
    xt_r = nc.dram_tensor("xt_r", [128, NT * NCT * TB], f32, kind="ExternalInput").ap()
    wq_r = nc.dram_tensor("wq_r", [128, NCT * GH * HD], f32, kind="ExternalInput").ap()
    wk_r = nc.dram_tensor("wk_r", [128, NCT * HD], f32, kind="ExternalInput").ap()
    wv_r = nc.dram_tensor("wv_r", [128, NCT * HD], f32, kind="ExternalInput").ap()
    wo_r = nc.dram_tensor("wo_r", [128, GH * HID], f32, kind="ExternalInput").ap()
    cos_r = nc.dram_tensor("cos_r", [128, S], f32, kind="ExternalInput").ap()
    sin_r = nc.dram_tensor("sin_r", [128, S], f32, kind="ExternalInput").ap()
    cb_r = nc.dram_tensor("cb_r", [128, 512], bf16, kind="ExternalInput").ap()
    u_r = nc.dram_tensor("u_r", [128, 512], bf16, kind="ExternalInput").ap()
    id_r = nc.dram_tensor("id_r", [128, 128], f32, kind="ExternalInput").ap()
    out_r = nc.dram_tensor("out_r", [32, NT * NCT * TB], f32, kind="ExternalOutput").ap()

    with tile.TileContext(nc) as tc:
        with (
            tc.tile_pool(name="glob", bufs=1) as glob,
            tc.tile_pool(name="wk", bufs=1) as wk,
            tc.tile_pool(name="dram", bufs=1, space="DRAM") as dram,
            tc.tile_pool(name="pchain", bufs=2, space="PSUM") as pchain,
            tc.tile_pool(name="pcho", bufs=2, space="PSUM") as pcho,
            tc.tile_pool(name="pscore", bufs=2, space="PSUM") as pscore,
            tc.tile_pool(name="pav", bufs=2, space="PSUM") as pav,
        ):
            # ---- persistent SBUF; DMA order matches block-0 consumption:
            # wq/xt chunks first (Q pass runs first), wk/wv behind, rope
            # tables and mask consts mid-stream, wo chunks after block 0 ----
            wq_all = glob.tile([128, NCT * GH * HD], f32r, tag="wq")  # [c-sub, ci*512+h*128+d]
            wk_all = glob.tile([128, NCT * HD], f32r, tag="wkt")      # [c-sub, ci*128+d]
            wv_all = glob.tile([128, NCT * HD], f32r, tag="wvt")
            cbs = glob.tile([128, 512], bf16, tag="cb")  # [U | negI | ones | -]
            U, negI, onesb = cbs[:, 0:128], cbs[:, 128:256], cbs[:, 256:384]
            U512 = glob.tile([128, 512], bf16, tag="u512")  # full-width mask pattern
            ident = glob.tile([128, 128], f32r, tag="id")
            cos0 = wk.tile([128, TB], f32, tag="cos", bufs=2, name="cos_t")
            sin0 = wk.tile([128, TB], f32, tag="sin", bufs=2, name="sin_t")
            xts0 = []
            for qc in range(4):
                nc.sync.dma_start(wq_all[:, qc * 2048:(qc + 1) * 2048],
                                  wq_r[:, qc * 2048:(qc + 1) * 2048].bitcast(f32r))
                for ci in range(4 * qc, 4 * qc + 4):
                    t0 = wk.tile([128, TB], f32r, tag="xt", bufs=18, name="xt")
                    nc.sync.dma_start(t0[:], xt_r[:, ci * TB:(ci + 1) * TB].bitcast(f32r))
                    xts0.append(t0)
                sl = slice(qc * 512, (qc + 1) * 512)
                nc.sync.dma_start(wk_all[:, sl], wk_r[:, sl].bitcast(f32r))
                nc.sync.dma_start(wv_all[:, sl], wv_r[:, sl].bitcast(f32r))
                if qc == 1:
                    nc.sync.dma_start(cos0[:], cos_r[:, 0:TB])
                    nc.sync.dma_start(sin0[:], sin_r[:, 0:TB])
                if qc == 2:
                    nc.sync.dma_start(cbs[:], cb_r[:])
                    nc.sync.dma_start(U512[:], u_r[:])
                    nc.sync.dma_start(ident[:], id_r[:].bitcast(f32r))
            wo_all = glob.tile([128, GH * HID], f32r, tag="wo")  # [d-sub, co*512+jh*128+c]
            for qc in range(4):
                nc.sync.dma_start(wo_all[:, qc * 2048:(qc + 1) * 2048],
                                  wo_r[:, qc * 2048:(qc + 1) * 2048].bitcast(f32r))

            kt_rope = glob.tile([128, S], f32r, tag="kt")             # roped K^T [d, k]
            v_all = glob.tile([128, S], bf16, tag="v")                # V natural, tile i at 128i

            if causal:
                qt = None  # per-block work tiles
            else:
                qt = [glob.tile([128, S], f32r, tag=f"qtg{h}", name=f"qt_g{h}")
                      for h in range(GH)]

            oT_part = dram.tile([128, NT * NCT * TB], f32)            # o^T partial (rearranged)
            oT_red = dram.tile([32, NT * NCT * TB], f32)

            def rope(ps, dst_ap, cos_t, sin_t):
                """dst = ps*cos + rot(ps)*sin_m (rotate_half sign in sin_m)."""
                raw = wk.tile([128, TB], f32, tag="raw", bufs=3, name="raw")
                nc.scalar.copy(raw[:], ps[:])
                rot = wk.tile([128, TB], f32, tag="rot", bufs=2, name="rot")
                nc.sync.dma_start(rot[0:64, :], raw[64:128, :])
                nc.sync.dma_start(rot[64:128, :], raw[0:64, :])
                m1 = wk.tile([128, TB], f32, tag="m1", bufs=2, name="m1")
                nc.vector.tensor_tensor(m1[:], raw[:], cos_t, op=MUL)
                m2 = wk.tile([128, TB], f32, tag="m2", bufs=2, name="m2")
                nc.vector.tensor_tensor(m2[:], rot[:], sin_t, op=MUL)
                nc.vector.tensor_tensor(dst_ap, m1[:], m2[:], op=ADD)

            def attention(j, qt_j):
                """Causal-aware attention for q block j + partial o_proj.
                Heads are processed singly: with po double-buffered, head
                h+1's AV accumulation starts while head h normalizes, so
                there is no inter-head pipeline bubble."""
                nkt = 4 * (j + 1) if causal else NKT
                at_s = []
                for h in range(GH):
                    ps_o = pav.tile([128, TB], f32, tag="po", name=f"ps_o{h}")
                    acc = wk.tile([128, TB], bf16, tag="acc", bufs=2, name=f"acc{h}")
                    for i in range(nkt):
                        m = i - 4 * j if causal else -1
                        a = 128 * m if m > 0 else 0          # valid q cols [a, TB)
                        qsl = qt_j[h] if a == 0 else qt_j[h][:, a:TB]
                        ps_s = pscore.tile([128, TB], f32, tag="ps", name="ps_s")
                        if m >= 0:
                            # additive causal mask goes FIRST with the start
                            # (=zeroing) bit: its operands are constants so it
                            # is always ready before the scores matmul below —
                            # the scheduler can never reorder the zeroing
                            # write after it.
                            nc.tensor.matmul(ps_s[:, a:TB], negI,
                                             U512[:, 0:TB - a],
                                             start=True, stop=False)
                        nc.tensor.matmul(ps_s[:, a:TB],
                                         kt_rope[:, 128 * i:128 * (i + 1)],
                                         qsl, start=(m < 0), stop=True)
                        if i == 0:
                            pt = acc
                            nc.scalar.activation(pt[:, a:TB], ps_s[:, a:TB], AF.Exp)
                        else:
                            pt = wk.tile([128, TB], bf16, tag="pt", bufs=5, name="pt")
                            nc.scalar.activation(pt[:, a:TB], ps_s[:, a:TB], AF.Exp)
                            nc.vector.tensor_tensor(acc[:, a:TB], acc[:, a:TB],
                                                    pt[:, a:TB], op=ADD)
                        nc.tensor.matmul(ps_o[:, a:TB],
                                         v_all[:, 128 * i:128 * (i + 1)], pt[:, a:TB],
                                         start=(i == 0), stop=(i == nkt - 1))
                    ps_d = pcho.tile([128, TB], f32, tag="cho", name="ps_d")
                    nc.tensor.matmul(ps_d[:], onesb, acc[:], start=True, stop=True)
                    rec = wk.tile([128, TB], f32, tag="rec", bufs=2, name="rec")
                    nc.vector.reciprocal(rec[:], ps_d[:])
                    at = wk.tile([128, TB], f32r, tag=f"at{h}", bufs=1, name=f"at{h}")
                    nc.vector.tensor_tensor(at[:], ps_o[:], rec[:], op=MUL)
                    at_s.append(at)

                # partial o_proj for q block j: 16 co chains of 4 matmuls.
                # For the last block the projection-chain pool is idle, so
                # alternate chains through it: 4 rotating banks hide the
                # PSUM->SBUF copy latency between chains.
                for cg in range(8):
                    ob = wk.tile([128, 1024], f32, tag="ob", bufs=3, name="ob")
                    for u in range(2):
                        co = 2 * cg + u
                        pool = pchain if (causal and j == NT - 1 and u == 1) else pcho
                        tg = "chain" if pool is pchain else "cho"
                        ps_p = pool.tile([128, TB], f32, tag=tg, name="ps_p")
                        for jh in range(GH):
                            nc.tensor.matmul(
                                ps_p[:],
                                wo_all[:, co * 512 + 128 * jh:co * 512 + 128 * (jh + 1)],
                                at_s[jh][:], start=(jh == 0), stop=(jh == GH - 1))
                        if u == 0:
                            nc.scalar.copy(ob[:, 0:512], ps_p[:])
                        else:
                            nc.vector.tensor_copy(ob[:, 512:1024], ps_p[:])
                    sl = slice(j * 8192 + cg * 1024, j * 8192 + (cg + 1) * 1024)
                    nc.sync.dma_start(oT_part[:, sl], ob[:])
                    if not with_rs:
                        # emit this core's output slice incrementally (the RS
                        # build instead reduces oT_part across the group)
                        nc.sync.dma_start(out_r[:, sl], ob[0:32, :])

            # ---- fused per-block pipeline ----
            for tb in range(NT):
                if tb == 0:
                    xts, cos_t, sin_t = xts0, cos0, sin0
                else:
                    xts = []
                    for ci in range(NCT):
                        t = wk.tile([128, TB], f32r, tag="xt", bufs=18, name="xt")
                        # issued via the idle Pool engine's SWDGE so these
                        # don't queue behind the weight loads on SP; the xt
                        # buffer rotation paces the transfers naturally
                        nc.gpsimd.dma_start(
                            t[:], xt_r[:, (tb * NCT + ci) * TB:(tb * NCT + ci + 1) * TB]
                            .bitcast(f32r))
                        xts.append(t)
                    cos_t = wk.tile([128, TB], f32, tag="cos", bufs=2, name="cos_t")
                    nc.sync.dma_start(cos_t[:], cos_r[:, TB * tb:TB * (tb + 1)])
                    sin_t = wk.tile([128, TB], f32, tag="sin", bufs=2, name="sin_t")
                    nc.sync.dma_start(sin_t[:], sin_r[:, TB * tb:TB * (tb + 1)])

                # passes 1+2: Q projection chains (pairs), rope into qt tiles.
                # Q first: attention's off-diagonal tiles only need this
                # block's Q (K/V come from earlier blocks), so it can start
                # while the K/V pass is still running.
                if causal:
                    qt_j = [wk.tile([128, TB], f32r, tag=f"qt{h}", bufs=2, name=f"qt{h}")
                            for h in range(GH)]
                else:
                    qt_j = None
                for qp in range(2):
                    ps_q = {}
                    for h in (2 * qp, 2 * qp + 1):
                        ps_q[h] = pchain.tile([128, TB], f32, tag="chain", name=f"ps_q{h}")
                    for ci in range(NCT):
                        st, sp = ci == 0, ci == NCT - 1
                        for h in (2 * qp, 2 * qp + 1):
                            nc.tensor.matmul(
                                ps_q[h][:],
                                wq_all[:, ci * 512 + 128 * h:ci * 512 + 128 * (h + 1)],
                                xts[ci][:], start=st, stop=sp)
                    for h in (2 * qp, 2 * qp + 1):
                        dst = qt_j[h][:] if causal else qt[h][:, TB * tb:TB * (tb + 1)]
                        rope(ps_q[h], dst, cos_t, sin_t)

                # pass 3: K and V projection chains
                ps_k = pchain.tile([128, TB], f32, tag="chain", name="ps_k")
                ps_v = pchain.tile([128, TB], f32, tag="chain", name="ps_v")
                for ci in range(NCT):
                    st, sp = ci == 0, ci == NCT - 1
                    nc.tensor.matmul(ps_k[:], wk_all[:, 128 * ci:128 * (ci + 1)],
                                     xts[ci][:], start=st, stop=sp)
                    nc.tensor.matmul(ps_v[:], wv_all[:, 128 * ci:128 * (ci + 1)],
                                     xts[ci][:], start=st, stop=sp)
                rope(ps_k, kt_rope[:, TB * tb:TB * (tb + 1)], cos_t, sin_t)
                vt_sb = wk.tile([128, TB], f32r, tag="vts", bufs=2, name="vt_sb")
                nc.vector.tensor_copy(vt_sb[:], ps_v[:])
                for u in range(TB // 128):
                    ps_tr = pchain.tile([128, 128], f32r, tag="chain", name="ps_tr")
                    nc.tensor.transpose(ps_tr[:], vt_sb[:, 128 * u:128 * (u + 1)],
                                        ident[:])
                    nc.scalar.copy(
                        v_all[:, 128 * (4 * tb + u):128 * (4 * tb + u + 1)],
                        ps_tr[:].bitcast(f32))

                if causal:
                    attention(tb, qt_j)

            if not causal:
                for j in range(NT):
                    attention(j, [q[:, TB * j:TB * (j + 1)] for q in qt])

            # ---- ReduceScatter partials, emit this core's slice ----
            if with_rs:
                nc.gpsimd.collective_compute(
                    "ReduceScatter", ADD,
                    replica_groups=[[0, 1, 2, 3], [4, 5, 6, 7]],
                    ins=[oT_part[:].opt()], outs=[oT_red[:].opt()],
                )
                nc.sync.dma_start(out_r[:], oT_red[:])

    nc.compile()
    return nc


def _host_arrange(a2d, nblk):
    """[nblk*128, F] -> [128, nblk*F] with block-major free dim."""
    n, fdim = a2d.shape
    assert n == nblk * 128
    return np.ascontiguousarray(
        a2d.reshape(nblk, 128, fdim).transpose(1, 0, 2).reshape(128, nblk * fdim))


def kernel(hidden_states, attention_mask, Wq, Wk, Wv, Wo, sin, cos):
    hidden_states = np.asarray(hidden_states, dtype=np.float32)
    attention_mask = np.asarray(attention_mask, dtype=np.float32)
    Wq, Wk, Wv, Wo = (np.ascontiguousarray(np.asarray(a, dtype=np.float32))
                      for a in (Wq, Wk, Wv, Wo))
    sin = np.asarray(sin, dtype=np.float32)
    cos = np.asarray(cos, dtype=np.float32)

    # classify the mask: causal (top-right strictly very-negative, elsewhere 0,
    # col 0 ignored since reference zeroes it) vs all-zeros (full attention)
    m0 = attention_mask[0, 0]
    iu = np.triu_indices(S, k=1)
    causal = bool((m0[iu] < -1e30).all() and
                  (m0[np.tril_indices(S, k=0)] == 0.0).all())
    if not causal:
        assert (attention_mask == 0).all(), "unsupported attention mask pattern"
    if causal:
        for b in range(1, B):
            assert np.array_equal(attention_mask[b, 0], m0), "mask differs per batch"

    key = causal
    if key not in _CACHE:
        _CACHE[key] = _build(causal)
    nc = _CACHE[key]

    cos_t = np.ascontiguousarray(cos[:S].T)          # [128, S]
    sin_m = np.ascontiguousarray(sin[:S].T)
    sin_m[:64] *= -1.0                               # rotate_half sign
    # bf16 const block: [U | negI | ones | unused]
    Um = (np.arange(512)[None, :] < np.arange(128)[:, None]).astype(np.float32)
    Um[:, 128:] = 0.0
    negI = np.eye(128, dtype=np.float32) * -32768.0
    onesb = np.ones((128, 128), dtype=np.float32)
    identb = np.eye(128, dtype=np.float32)
    cb = np.concatenate([Um[:, :128], negI, onesb, np.zeros((128, 128), np.float32)],
                        axis=1).astype(ml_dtypes.bfloat16)
    u512 = Um.astype(ml_dtypes.bfloat16)

    in_maps = []
    for c in range(8):
        b, g = c // 4, c % 4
        xt = np.ascontiguousarray(hidden_states[b].T)          # [HID, S]
        # [128, tb*16*512 + ci*512 + col] <- xt[128*ci+p, 512*tb+col]
        xt_r = np.ascontiguousarray(
            xt.reshape(NCT, 128, NT, TB).transpose(1, 2, 0, 3).reshape(128, NT * NCT * TB))
        wq_r = _host_arrange(Wq[512 * g:512 * (g + 1), :].T, NCT)
        wk_r = _host_arrange(Wk[128 * g:128 * (g + 1), :].T, NCT)
        wv_r = _host_arrange(Wv[128 * g:128 * (g + 1), :].T, NCT)
        # co-major o_proj weights: wo_r[p, co*512+jh*128+cc] = Wo[128co+cc, 512g+128jh+p]
        wo_r = np.ascontiguousarray(
            Wo[:, 512 * g:512 * (g + 1)].reshape(NCT, 128, GH, 128)
            .transpose(3, 0, 2, 1).reshape(128, GH * HID))
        in_maps.append({
            "xt_r": xt_r, "wq_r": wq_r, "wk_r": wk_r, "wv_r": wv_r, "wo_r": wo_r,
            "cos_r": cos_t, "sin_r": sin_m, "cb_r": cb, "u_r": u512, "id_r": identb,
        })

    global _LAST_IN_MAPS
    _LAST_IN_MAPS = in_maps
    res = run_bass_kernel_spmd(nc, in_maps, core_ids=list(range(8)))

    out = np.empty((B, S, HID), dtype=np.float32)
    obuf = np.empty((NCT, 128, NT, TB), dtype=np.float32)  # [co, p, j, col]
    for b in range(B):
        for r in range(4):
            part = res.results[4 * b + r]["out_r"]          # [32, j*8192+co*512+col]
            obuf[:, 32 * r:32 * (r + 1)] = (
                part.reshape(32, NT, NCT, TB).transpose(2, 0, 1, 3))
        out[b] = obuf.reshape(HID, S).T
    return out


if __name__ == "__main__":
    print("module loads ok")
